# revision 1
# baseline (speedup 1.0000x reference)
"""Two-layer GAT (nn_GAT_82334523064895) on 8 TRN2 NeuronCores via Bass.

Strategy (8-way contiguous node sharding, SPMD single NEFF):
  1. h_aug = x_shard @ [W1 | W1@a_s1 | W1@a_d1] in bf16; x loaded k-major via
     DMA-transpose, PE accumulates h.T in PSUM over 64 k-chunks.
  2. h rows written to a 256B-row padded table, AllGather -> replicated table.
  3. Edge phase as padded ELL (no scatter): per 128-dst tile one dma_gather
     fetches h_aug[src] rows (slot 0 = self loop; pad slots hit a sentinel row
     whose s-value = -3e4 so exp -> exactly 0). Softmax without max-shift
     (edge logits are O(1)), weighted sums on DVE.
  4. Layer-2 (C=2) via DVE matvec, AllGather of table 2, same edge phase.
  5. Global min/max via AllReduce(max) of [max, -min]; rescale on device.
Host does only integer index prep, sharding, dtype casts, and unpermute.
"""

import numpy as np
import ml_dtypes

N = 8192
F = 8192
H = 16
C = 2
NCORES = 8
NSH = N // NCORES          # nodes per core
P = 128
NT = NSH // P              # dst tiles per core
AUG1 = H + 2               # h .. s, d
KCH = F // P               # k chunks
ROW = 64                   # f32 elements per padded table row (256B)
NSHE = NSH + 1             # slab rows: NSH nodes + 1 sentinel row
PAD = N                    # sentinel marker in raw (node-id) index space
PADPOS = NSH               # sentinel position within core-0 block of the table
TROWS = NCORES * NSHE
NEG = 0.2
SENT = -30000.0


class _PhaseDone(Exception):
    pass


def _install_tilefix():
    """Split the Tile kernel-tail drain's sem waits across multiple drains
    (this walrus build rejects >1 sync wait on a CTRL instruction)."""
    import bass_rust
    from bass_rust import ScopedClock
    import concourse.tile as tile

    def _split_drain_and_barrier(self, tick_clock, wait_clock):
        nc = self.nc
        drain_inst = nc.sync.drain()
        wait_clock.add_sem_waits(
            drain_inst.ins, ScopedClock({None: tick_clock.global_clock})
        )
        si = drain_inst.ins.sync_info
        waits = list(si.on_wait) if si is not None else []
        if len(waits) > 1:
            si.on_wait = waits[:1]
            for i in range(1, len(waits)):
                d2 = nc.sync.drain()
                si2 = d2.ins.sync_info
                if si2 is None:
                    d2.ins.sync_info = bass_rust.SyncInfo(on_wait=[], on_update=[])
                    si2 = d2.ins.sync_info
                si2.on_wait = waits[i : i + 1]
        nc.all_engine_barrier()
        popped = nc._tile_sem_poison_stack.pop()
        assert popped is self._sem_poison
        nc.clear_and_free_semaphores(list(self.sems.allocated().values()))
        nc.all_engine_barrier()

    tile.TileContext._drain_and_barrier = _split_drain_and_barrier


def _split_multiwaits(d):
    """Walrus in this build accepts a single sync wait per instruction; hoist
    extra waits onto wait-only EventSemaphore carriers inserted just before."""
    n = 0
    for fn in d["functions"]:
        for blk in fn["blocks"]:
            newl = []
            for ins in blk["instructions"]:
                si = ins.get("sync_info")
                waits = (si or {}).get("on_wait") or []
                if len(waits) > 1:
                    for w in waits[:-1]:
                        n += 1
                        newl.append(
                            {
                                "debug": ins.get("debug"),
                                "engine": ins["engine"],
                                "ins": [],
                                "outs": [],
                                "name": f"{ins['name']}-ws{n}",
                                "opcode": "EventSemaphore",
                                "sync_info": {"on_update": [], "on_wait": [w]},
                            }
                        )
                    si["on_wait"] = [waits[-1]]
                newl.append(ins)
            blk["instructions"] = newl
    return d


def _patch_serialization(nc):
    import types
    import json

    orig = nc.to_json_bytes

    def to_json_bytes_patched(self):
        d = json.loads(orig())
        _split_multiwaits(d)
        return json.dumps(d).encode()

    nc.to_json_bytes = types.MethodType(to_json_bytes_patched, nc)


def _build(wts, phase="full"):
    import concourse.bass as bass
    import concourse.bacc as bacc
    import concourse.mybir as mybir
    import concourse.tile as tile
    from concourse.masks import make_identity
    import bass_rust

    _install_tilefix()
    dt = mybir.dt
    Alu = mybir.AluOpType
    Act = mybir.ActivationFunctionType
    RG = [list(range(NCORES))]

    wts = list(wts)
    IW = 8 * sum(wts)

    nc = bacc.Bacc("TRN2", debug=False)
    xs_p = nc.declare_dram_parameter("xs", [NSH, F], dt.bfloat16, isOutput=False)
    w1_p = nc.declare_dram_parameter("w1s", [P, KCH * AUG1], dt.bfloat16, isOutput=False)
    v1_p = nc.declare_dram_parameter("v1", [AUG1, 1], dt.float32, isOutput=False)
    idx1_p = nc.declare_dram_parameter("idx1", [P, IW], dt.int16, isOutput=False)
    idx2_p = nc.declare_dram_parameter("idx2", [P, IW], dt.int16, isOutput=False)
    w2b_p = nc.declare_dram_parameter("w2b", [P, 4 * H], dt.float32, isOutput=False)
    b2a_p = nc.declare_dram_parameter("b2a", [P, 4], dt.float32, isOutput=False)
    out_p = nc.declare_dram_parameter("out", [NSH, C], dt.float32, isOutput=True)

    with tile.TileContext(nc) as tc:
        with (
            tc.tile_pool(name="const", bufs=1) as cpool,
            tc.tile_pool(name="xload", bufs=4) as xpool,
            tc.tile_pool(name="work", bufs=3) as wpool,
            tc.tile_pool(name="gath", bufs=2) as gpool,
            tc.tile_pool(name="pst", bufs=2, space="PSUM") as ppool,
            tc.tile_pool(name="psacc", bufs=1, space="PSUM") as psacc,
            tc.tile_pool(name="dram", bufs=1, space="DRAM") as dpool,
        ):
            def _emit():
                # ---- constants
                w1_s = cpool.tile([P, KCH, AUG1], dt.bfloat16)
                nc.sync.dma_start(w1_s[:], w1_p[:].rearrange("p (c f) -> p c f", f=AUG1))
                v1_s = cpool.tile([AUG1, 1], dt.float32)
                nc.sync.dma_start(v1_s[:], v1_p[:])
                idx1_s = cpool.tile([P, IW], dt.int16)
                nc.sync.dma_start(idx1_s[:], idx1_p[:])
                idx2_s = cpool.tile([P, IW], dt.int16)
                nc.sync.dma_start(idx2_s[:], idx2_p[:])
                w2b_s = cpool.tile([P, 4, H], dt.float32)
                nc.sync.dma_start(w2b_s[:], w2b_p[:].rearrange("p (c k) -> p c k", k=H))
                b2a_s = cpool.tile([P, 4], dt.float32)
                nc.sync.dma_start(b2a_s[:], b2a_p[:])
                ident = cpool.tile([P, P], dt.float32)
                make_identity(nc, ident[:])
                sent = cpool.tile([1, ROW], dt.float32)
                nc.gpsimd.memset(sent[:], SENT)

                # ---- internal DRAM
                l1slab = dpool.tile([NSHE, ROW], dt.float32)
                table1 = dpool.tile([TROWS, ROW], dt.float32, addr_space="Shared")
                l2slab = dpool.tile([NSHE, ROW], dt.float32)
                table2 = dpool.tile([TROWS, ROW], dt.float32, addr_space="Shared")
                mmx_i = dpool.tile([1, 2], dt.float32)
                mmx_o = dpool.tile([1, 2], dt.float32, addr_space="Shared")

                # sentinel row rides along in each core's slab (Shared tables may
                # only be written by the collective itself)
                nc.sync.dma_start(l1slab[NSH : NSH + 1, :], sent[:])
                nc.sync.dma_start(l2slab[NSH : NSH + 1, :], sent[:])
                # zero-fill the pad columns the h-row writes don't cover
                z64 = cpool.tile([P, ROW], dt.float32)
                nc.gpsimd.memset(z64[:], 0.0)
                for t in range(NT):
                    nc.sync.dma_start(l1slab[t * P : (t + 1) * P, :], z64[:])
                    nc.sync.dma_start(l2slab[t * P : (t + 1) * P, :], z64[:])

                # ---- Phase A: hT = W1aug.T @ x.T accumulated over k chunks
                hps0 = psacc.tile([AUG1, 512], dt.float32, tag="hps0")
                hps1 = psacc.tile([AUG1, 512], dt.float32, tag="hps1")
                hps = [hps0, hps1]
                for ck in range(KCH):
                    xt = xpool.tile([P, 1024], dt.bfloat16, tag="xt")
                    for mh in range(2):
                        nc.sync.dma_start_transpose(
                            xt[:, mh * 512 : (mh + 1) * 512],
                            xs_p[mh * 512 : (mh + 1) * 512, ck * P : (ck + 1) * P],
                        )
                    for mh in range(2):
                        nc.tensor.matmul(
                            hps[mh][:],
                            w1_s[:, ck, :],
                            xt[:, mh * 512 : (mh + 1) * 512],
                            start=(ck == 0),
                            stop=(ck == KCH - 1),
                        )
                hT = cpool.tile([AUG1, NSH], dt.float32)
                for mh in range(2):
                    nc.scalar.activation(
                        hT[:, mh * 512 : (mh + 1) * 512],
                        hps[mh][:],
                        Act.Identity,
                        bias=v1_s[:],
                    )

                # ---- Phase B: h rows -> l1slab -> AllGather table1
                for t in range(NT):
                    hr_ps = ppool.tile([P, AUG1], dt.float32, tag="hrps")
                    nc.tensor.transpose(
                        hr_ps[:], hT[:, t * P : (t + 1) * P], ident[:AUG1, :AUG1]
                    )
                    hr = wpool.tile([P, AUG1], dt.float32, tag="hr")
                    nc.vector.tensor_copy(hr[:], hr_ps[:])
                    nc.sync.dma_start(l1slab[t * P : (t + 1) * P, 0:AUG1], hr[:])
                nc.gpsimd.collective_compute(
                    "AllGather",
                    Alu.bypass,
                    replica_groups=RG,
                    ins=[l1slab[:].opt()],
                    outs=[table1[0 : NCORES * NSHE, :].opt()],
                )

                allout = cpool.tile([P, NT, C], dt.float32)

                def edge_layer(table, idx_s, FH, scol, dcol, layer):
                    off = 0
                    for t in range(NT):
                        Wt = wts[t]
                        ni = P * Wt
                        G = gpool.tile([P, Wt, ROW], dt.float32, tag="G")
                        nc.gpsimd.dma_gather(
                            out_ap=G[:],
                            in_ap=table[:],
                            idxs_ap=idx_s[:, off : off + 8 * Wt],
                            num_idxs=ni,
                            num_idxs_reg=ni,
                            elem_size=ROW,
                            single_packet=False,
                        )
                        off += 8 * Wt
                        if phase == "gonly":
                            gq = wpool.tile([P, C], dt.float32, tag="gq")
                            nc.vector.tensor_copy(gq[:], G[:, 0, 0:C])
                            nc.sync.dma_start(
                                out_p[t * P : (t + 1) * P, :], gq[:]
                            )
                            continue
                        # z = s[src] + d[dst(self)]
                        z = wpool.tile([P, Wt], dt.float32, tag="z")
                        nc.scalar.activation(
                            z[:],
                            G[:, 0:Wt, scol : scol + 1].squeeze(),
                            Act.Identity,
                            bias=G[:, 0:1, dcol : dcol + 1].rearrange("p a b -> p (a b)"),
                        )
                        # e = max(z, 0.2 z)  (leaky relu)
                        e = wpool.tile([P, Wt], dt.float32, tag="e")
                        nc.vector.scalar_tensor_tensor(
                            out=e[:], in0=z[:], scalar=NEG, in1=z[:],
                            op0=Alu.mult, op1=Alu.max,
                        )
                        # ex = exp(e), den = sum(ex)
                        ex = wpool.tile([P, Wt], dt.float32, tag="ex")
                        den = wpool.tile([P, 1], dt.float32, tag="den")
                        nc.scalar.activation(ex[:], e[:], Act.Exp, accum_out=den[:])
                        rec = wpool.tile([P, 1], dt.float32, tag="rec")
                        nc.vector.reciprocal(rec[:], den[:])
                        # num[p,f] = sum_s ex[p,s] * G[p,s,f]
                        tmp = wpool.tile([P, FH, Wt], dt.float32, tag=f"tmp{layer}")
                        nc.vector.tensor_tensor(
                            out=tmp[:],
                            in0=G[:, 0:Wt, 0:FH].rearrange("p s f -> p f s"),
                            in1=ex[:].unsqueeze(1).to_broadcast([P, FH, Wt]),
                            op=Alu.mult,
                        )
                        num = wpool.tile([P, FH], dt.float32, tag=f"num{layer}")
                        nc.vector.tensor_reduce(
                            num[:], tmp[:], axis=mybir.AxisListType.X, op=Alu.add
                        )
                        if layer == 1:
                            o1 = wpool.tile([P, H], dt.float32, tag="o1")
                            nc.vector.tensor_scalar_mul(o1[:], num[:], rec[:])
                            # h2_aug = o1 @ W2aug (+b2 pattern) on DVE
                            tmp2 = wpool.tile([P, 4, H], dt.float32, tag="tmp2")
                            nc.vector.tensor_tensor(
                                out=tmp2[:],
                                in0=o1[:].unsqueeze(1).to_broadcast([P, 4, H]),
                                in1=w2b_s[:],
                                op=Alu.mult,
                            )
                            h2t = wpool.tile([P, 4], dt.float32, tag="h2t")
                            nc.vector.tensor_reduce(
                                h2t[:], tmp2[:], axis=mybir.AxisListType.X, op=Alu.add
                            )
                            h2b = wpool.tile([P, 4], dt.float32, tag="h2b")
                            nc.vector.tensor_add(h2b[:], h2t[:], b2a_s[:])
                            nc.sync.dma_start(
                                l2slab[t * P : (t + 1) * P, 0:4], h2b[:]
                            )
                        else:
                            nc.vector.tensor_scalar_mul(
                                allout[:, t, :], num[:], rec[:]
                            )

                # ---- Phase C: layer-1 edge aggregation + h2
                if phase == "gemm":
                    nc.sync.dma_start(out_p[:], l1slab[0:NSH, 0:C])
                    return
                if phase == "ag1":
                    nc.sync.dma_start(out_p[:], table1[0:NSH, 0:C])
                    return
                edge_layer(table1, idx1_s, H, H, H + 1, 1)
                if phase == "gonly":
                    return
                if phase == "gat1":
                    nc.sync.dma_start(out_p[:], l2slab[0:NSH, 0:C])
                    return
                nc.gpsimd.collective_compute(
                    "AllGather",
                    Alu.bypass,
                    replica_groups=RG,
                    ins=[l2slab[:].opt()],
                    outs=[table2[0 : NCORES * NSHE, :].opt()],
                )
                if phase == "ag2":
                    nc.sync.dma_start(out_p[:], table2[0:NSH, 0:C])
                    return
                # ---- Phase D: layer-2 edge aggregation
                edge_layer(table2, idx2_s, C, C, C + 1, 2)
                if phase == "gat2":
                    for t in range(NT):
                        nc.sync.dma_start(out_p[t * P : (t + 1) * P, :], allout[:, t, :])
                    return

                # ---- Phase E: global min/max + rescale
                mm = wpool.tile([P, 2], dt.float32, tag="mm")
                nc.vector.tensor_reduce(
                    mm[:, 0:1], allout[:], axis=mybir.AxisListType.XY, op=Alu.max
                )
                mnt = wpool.tile([P, 1], dt.float32, tag="mnt")
                nc.vector.tensor_reduce(
                    mnt[:], allout[:], axis=mybir.AxisListType.XY, op=Alu.min
                )
                nc.vector.tensor_scalar_mul(mm[:, 1:2], mnt[:], -1.0)
                pr = wpool.tile([P, 2], dt.float32, tag="pr")
                nc.gpsimd.partition_all_reduce(
                    pr[:], mm[:], channels=P, reduce_op=bass_rust.ReduceOp.max
                )
                nc.sync.dma_start(mmx_i[:], pr[0:1, :])
                nc.gpsimd.collective_compute(
                    "AllReduce",
                    Alu.max,
                    replica_groups=RG,
                    ins=[mmx_i[:].opt()],
                    outs=[mmx_o[:].opt()],
                )
                mmr = wpool.tile([1, 2], dt.float32, tag="mmr")
                nc.sync.dma_start(mmr[:], mmx_o[:])
                bc = wpool.tile([P, 2], dt.float32, tag="bc")
                nc.gpsimd.partition_broadcast(bc[:], mmr[:])
                # scale = 2/(mx-mn); shift = 2*(-mn)/(mx-mn) - 1
                rng_ = wpool.tile([P, 1], dt.float32, tag="rng")
                nc.vector.tensor_tensor(rng_[:], bc[:, 0:1], bc[:, 1:2], op=Alu.add)
                ri = wpool.tile([P, 1], dt.float32, tag="ri")
                nc.vector.reciprocal(ri[:], rng_[:])
                sc = wpool.tile([P, 1], dt.float32, tag="sc")
                nc.vector.tensor_scalar_mul(sc[:], ri[:], 2.0)
                u = wpool.tile([P, 1], dt.float32, tag="u")
                nc.vector.tensor_tensor(u[:], bc[:, 1:2], ri[:], op=Alu.mult)
                sh = wpool.tile([P, 1], dt.float32, tag="sh")
                nc.vector.tensor_scalar(
                    out=sh[:], in0=u[:], scalar1=2.0, scalar2=-1.0,
                    op0=Alu.mult, op1=Alu.add,
                )
                for t in range(NT):
                    fin = wpool.tile([P, C], dt.float32, tag="fin")
                    nc.vector.tensor_scalar(
                        out=fin[:], in0=allout[:, t, :], scalar1=sc[:], scalar2=sh[:],
                        op0=Alu.mult, op1=Alu.add,
                    )
                    nc.sync.dma_start(out_p[t * P : (t + 1) * P, :], fin[:])

            _emit()
    nc.compile()
    _patch_serialization(nc)
    return nc


def _prep(x, edge_index, W1, a_src1, a_dst1, b1, W2, a_src2, a_dst2, b2):
    ei = np.asarray(edge_index).astype(np.int64)
    src_all, dst_all = ei[0], ei[1]
    counts = np.bincount(dst_all, minlength=N)
    perm_e = np.argsort(dst_all, kind="stable")
    ssorted = src_all[perm_e].astype(np.int64)
    starts = np.zeros(N + 1, np.int64)
    np.cumsum(counts, out=starts[1:])

    orders = []
    wt_core = np.zeros((NCORES, NT), np.int64)
    for c in range(NCORES):
        ids = np.arange(NSH * c, NSH * (c + 1))
        o = ids[np.argsort(-counts[ids], kind="stable")]
        orders.append(o)
        for t in range(NT):
            wt_core[c, t] = 1 + counts[o[P * t]]
    wts = tuple(int(w) for w in wt_core.max(axis=0))

    # table positions: node g lives at NSHE*core + local; sentinel at PADPOS
    pos1 = np.empty(N + 1, np.int64)
    g = np.arange(N)
    pos1[g] = NSHE * (g // NSH) + (g % NSH)
    pos1[PAD] = PADPOS
    pos2 = np.empty(N + 1, np.int64)
    pos2[PAD] = PADPOS
    for c in range(NCORES):
        pos2[orders[c]] = NSHE * c + np.arange(NSH)

    idx1_maps, idx2_maps = [], []
    for c in range(NCORES):
        segs1 = []
        for t in range(NT):
            wt = wts[t]
            nodes = orders[c][P * t : P * (t + 1)]
            mat = np.full((wt, P), PAD, np.int64)
            mat[0, :] = nodes
            for p, g in enumerate(nodes):
                dg = counts[g]
                mat[1 : 1 + dg, p] = ssorted[starts[g] : starts[g] + dg]
            segs1.append(mat)
        raw = np.concatenate([m.reshape(-1) for m in segs1])
        unwrap1 = pos1[raw]
        unwrap2 = pos2[raw]

        def wrap(unwrap):
            parts = []
            o = 0
            for t in range(NT):
                n = P * wts[t]
                parts.append(unwrap[o : o + n].reshape(-1, 16).T)
                o += n
            w16 = np.concatenate(parts, axis=1).astype(np.int16)
            return np.tile(w16, (NCORES, 1))

        idx1_maps.append(wrap(unwrap1))
        idx2_maps.append(wrap(unwrap2))

    bf = ml_dtypes.bfloat16
    W1aug = np.concatenate(
        [W1, (W1 @ a_src1)[:, None], (W1 @ a_dst1)[:, None]], axis=1
    ).astype(np.float32)
    w1s = (
        W1aug.reshape(KCH, P, AUG1).transpose(1, 0, 2).reshape(P, KCH * AUG1)
    ).astype(bf)
    v1 = np.concatenate([b1.astype(np.float32), np.zeros(2, np.float32)]).reshape(
        AUG1, 1
    )
    W2aug = np.concatenate(
        [W2, (W2 @ a_src2)[:, None], (W2 @ a_dst2)[:, None]], axis=1
    ).astype(np.float32)
    w2b = np.tile(W2aug.T.reshape(1, 4 * H), (P, 1)).astype(np.float32)
    b2a = np.tile(
        np.array([b2[0], b2[1], 0.0, 0.0], np.float32), (P, 1)
    ).astype(np.float32)

    x = np.asarray(x, np.float32)
    in_maps = []
    for c in range(NCORES):
        in_maps.append(
            {
                "xs": np.ascontiguousarray(x[NSH * c : NSH * (c + 1)]).astype(bf),
                "w1s": w1s,
                "v1": v1,
                "idx1": idx1_maps[c],
                "idx2": idx2_maps[c],
                "w2b": w2b,
                "b2a": b2a,
            }
        )
    return wts, in_maps, orders


_NC_CACHE = {}


def _get_nc(wts):
    if wts not in _NC_CACHE:
        _NC_CACHE[wts] = _build(wts)
    return _NC_CACHE[wts]


def kernel(**inputs):
    from concourse.bass_utils import run_bass_kernel_spmd

    wts, in_maps, orders = _prep(
        inputs["x"], inputs["edge_index"], inputs["W1"], inputs["a_src1"],
        inputs["a_dst1"], inputs["b1"], inputs["W2"], inputs["a_src2"],
        inputs["a_dst2"], inputs["b2"],
    )
    nc = _get_nc(wts)
    res = run_bass_kernel_spmd(nc, in_maps, list(range(NCORES)))
    out = np.empty((N, C), np.float32)
    for c in range(NCORES):
        out[orders[c]] = res.results[c]["out"]
    return out



# revision 5
# speedup vs baseline: 1.9016x; 1.9016x over previous
"""Two-layer GAT (nn_GAT_82334523064895) on 8 TRN2 NeuronCores via Bass.

Strategy (8-way contiguous node sharding, SPMD single NEFF):
  1. x is transposed + bf16-cast on host so phase A streams it at line rate
     (no DMA-transpose): hT = W1aug.T @ xT accumulated in PSUM over 64
     k-chunks, W1aug = [W1 | W1@a_s1 | W1@a_d1].
  2. h rows packed as 40B table rows [16*bf16 h | f32 s | f32 d] into a
     DRAM slab; small AllGather (328 KB) replicates the table.
  3. Edge phase as uniform-width padded ELL (W = 1 + global max in-degree):
     ONE indirect-DMA gather per layer fetches all h[src] rows
     ([128, NT, W, row]); pad slots hit a sentinel row (s = -3e4 so
     exp -> 0). Softmax without max-shift; weighted sums on DVE as a
     handful of fused whole-shard ops (no per-tile loop).
  4. Layer 2 identically with 16B f32 rows [o2c0 | o2c1 | s2 | d2]
     (h2aug = o1 @ W2aug on DVE; b2 folded in pre-aggregation, valid
     since sum(alpha) = 1).
  5. Unnormalized outputs returned; global min/max rescale done on host.
Host does index prep (shared by both layers), x transpose/cast, rescale.
"""

import numpy as np
import ml_dtypes

N = 8192
F = 8192
H = 16
C = 2
NCORES = 8
NSH = N // NCORES          # nodes per core
P = 128
NT = NSH // P              # dst tiles per core
AUG1 = H + 2               # h .. s, d
KCH = F // P               # k chunks
ROW1 = 10                  # f32 elems per L1 table row (16 bf16 + 2 f32)
ROW2 = 4                   # f32 elems per L2 table row
NSHE = NSH + 1             # slab rows: NSH nodes + 1 sentinel row
TROWS = NCORES * NSHE
SENTROW = NSH              # sentinel position = core-0 slab row NSH
NEG = 0.2
SENT = -30000.0


class _PhaseDone(Exception):
    pass


def _install_tilefix():
    """Split the Tile kernel-tail drain's sem waits across multiple drains
    (this walrus build rejects >1 sync wait on a CTRL instruction)."""
    import bass_rust
    from bass_rust import ScopedClock
    import concourse.tile as tile

    def _split_drain_and_barrier(self, tick_clock, wait_clock):
        nc = self.nc
        drain_inst = nc.sync.drain()
        wait_clock.add_sem_waits(
            drain_inst.ins, ScopedClock({None: tick_clock.global_clock})
        )
        si = drain_inst.ins.sync_info
        waits = list(si.on_wait) if si is not None else []
        if len(waits) > 1:
            si.on_wait = waits[:1]
            for i in range(1, len(waits)):
                d2 = nc.sync.drain()
                si2 = d2.ins.sync_info
                if si2 is None:
                    d2.ins.sync_info = bass_rust.SyncInfo(on_wait=[], on_update=[])
                    si2 = d2.ins.sync_info
                si2.on_wait = waits[i : i + 1]
        nc.all_engine_barrier()
        popped = nc._tile_sem_poison_stack.pop()
        assert popped is self._sem_poison
        nc.clear_and_free_semaphores(list(self.sems.allocated().values()))
        nc.all_engine_barrier()

    tile.TileContext._drain_and_barrier = _split_drain_and_barrier


def _split_multiwaits(d):
    """Walrus in this build accepts a single sync wait per instruction; hoist
    extra waits onto wait-only EventSemaphore carriers inserted just before."""
    n = 0
    for fn in d["functions"]:
        for blk in fn["blocks"]:
            newl = []
            for ins in blk["instructions"]:
                si = ins.get("sync_info")
                waits = (si or {}).get("on_wait") or []
                if len(waits) > 1:
                    for w in waits[:-1]:
                        n += 1
                        newl.append(
                            {
                                "debug": ins.get("debug"),
                                "engine": ins["engine"],
                                "ins": [],
                                "outs": [],
                                "name": f"{ins['name']}-ws{n}",
                                "opcode": "EventSemaphore",
                                "sync_info": {"on_update": [], "on_wait": [w]},
                            }
                        )
                    si["on_wait"] = [waits[-1]]
                newl.append(ins)
            blk["instructions"] = newl
    return d


def _patch_serialization(nc):
    import types
    import json

    orig = nc.to_json_bytes

    def to_json_bytes_patched(self):
        d = json.loads(orig())
        _split_multiwaits(d)
        return json.dumps(d).encode()

    nc.to_json_bytes = types.MethodType(to_json_bytes_patched, nc)


def _build(W, phase="full"):
    import concourse.bass as bass
    import concourse.bacc as bacc
    import concourse.mybir as mybir
    import concourse.tile as tile
    from concourse.masks import make_identity

    _install_tilefix()
    dt = mybir.dt
    Alu = mybir.AluOpType
    Act = mybir.ActivationFunctionType
    RG = [list(range(NCORES))]
    NW = NT * W

    nc = bacc.Bacc("TRN2", debug=False)
    xs_p = nc.declare_dram_parameter("xs", [F, NSH], dt.bfloat16, isOutput=False)
    w1_p = nc.declare_dram_parameter("w1s", [P, KCH * AUG1], dt.bfloat16, isOutput=False)
    v1_p = nc.declare_dram_parameter("v1", [AUG1, 1], dt.float32, isOutput=False)
    idx_p = nc.declare_dram_parameter("idx", [P, NW], dt.int32, isOutput=False)
    w2b_p = nc.declare_dram_parameter("w2b", [P, 4 * H], dt.float32, isOutput=False)
    b2a_p = nc.declare_dram_parameter("b2a", [P, 4], dt.float32, isOutput=False)
    out_p = nc.declare_dram_parameter("out", [NSH, C], dt.float32, isOutput=True)

    with tile.TileContext(nc) as tc:
        with (
            tc.tile_pool(name="const", bufs=1) as cpool,
            tc.tile_pool(name="xload", bufs=4) as xpool,
            tc.tile_pool(name="work", bufs=1) as wpool,
            tc.tile_pool(name="gath", bufs=1) as gpool,
            tc.tile_pool(name="pst", bufs=2, space="PSUM") as ppool,
            tc.tile_pool(name="psacc", bufs=1, space="PSUM") as psacc,
            tc.tile_pool(name="dram", bufs=1, space="DRAM") as dpool,
        ):
            def _emit():
                # ---- constants
                w1_s = cpool.tile([P, KCH, AUG1], dt.bfloat16)
                nc.sync.dma_start(w1_s[:], w1_p[:].rearrange("p (c f) -> p c f", f=AUG1))
                v1_s = cpool.tile([AUG1, 1], dt.float32)
                nc.sync.dma_start(v1_s[:], v1_p[:])
                idx_s = cpool.tile([P, NW], dt.int32)
                nc.sync.dma_start(idx_s[:], idx_p[:])
                w2b_s = cpool.tile([P, 4, H], dt.float32)
                nc.sync.dma_start(w2b_s[:], w2b_p[:].rearrange("p (c k) -> p c k", k=H))
                b2a_s = cpool.tile([P, 4], dt.float32)
                nc.sync.dma_start(b2a_s[:], b2a_p[:])
                ident = cpool.tile([P, P], dt.float32)
                make_identity(nc, ident[:])
                sent1 = cpool.tile([1, ROW1], dt.float32)
                nc.gpsimd.memset(sent1[:], SENT)
                sent2 = cpool.tile([1, ROW2], dt.float32)
                nc.gpsimd.memset(sent2[:], SENT)

                # ---- internal DRAM
                l1slab = dpool.tile([NSHE, ROW1], dt.float32)
                table1 = dpool.tile([TROWS, ROW1], dt.float32, addr_space="Shared")
                l2slab = dpool.tile([NSHE, ROW2], dt.float32)
                table2 = dpool.tile([TROWS, ROW2], dt.float32, addr_space="Shared")

                # ---- Phase A: hT = W1aug.T @ xT accumulated over k chunks
                hps0 = psacc.tile([AUG1, 512], dt.float32, tag="hps0")
                hps1 = psacc.tile([AUG1, 512], dt.float32, tag="hps1")
                hps = [hps0, hps1]
                for ck in range(KCH // 2):
                    xt = xpool.tile([P, 2, NSH], dt.bfloat16, tag="xt")
                    nc.sync.dma_start(
                        xt[:],
                        xs_p[ck * 2 * P : (ck + 1) * 2 * P, :].rearrange(
                            "(j p) n -> p j n", p=P
                        ),
                    )
                    for j in range(2):
                        c2 = 2 * ck + j
                        for mh in range(2):
                            nc.tensor.matmul(
                                hps[mh][:],
                                w1_s[:, c2, :],
                                xt[:, j, mh * 512 : (mh + 1) * 512],
                                start=(c2 == 0),
                                stop=(c2 == KCH - 1),
                            )
                hT = cpool.tile([AUG1, NSH], dt.float32)
                for mh in range(2):
                    nc.scalar.activation(
                        hT[:, mh * 512 : (mh + 1) * 512],
                        hps[mh][:],
                        Act.Identity,
                        bias=v1_s[:],
                    )

                # ---- Phase B: pack h rows [16 bf16 | s f32 | d f32] -> slab
                slabsb = cpool.tile([P, NT, ROW1], dt.float32)
                slabbf = slabsb.bitcast(dt.bfloat16)  # [P, NT, 2*ROW1]
                for t in range(NT):
                    hr_ps = ppool.tile([P, AUG1], dt.float32, tag="hrps")
                    nc.tensor.transpose(
                        hr_ps[:], hT[:, t * P : (t + 1) * P], ident[:AUG1, :AUG1]
                    )
                    nc.vector.tensor_copy(slabbf[:, t, 0:H], hr_ps[:, 0:H])
                    nc.vector.tensor_copy(slabsb[:, t, H // 2 : ROW1], hr_ps[:, H : AUG1])
                nc.sync.dma_start(
                    l1slab[0:NSH, :].rearrange("(t p) f -> p t f", p=P), slabsb[:]
                )
                nc.sync.dma_start(l1slab[NSH : NSH + 1, :], sent1[:])
                if phase == "gemm":
                    nc.sync.dma_start(out_p[:], l1slab[0:NSH, 0:C])
                    return
                nc.gpsimd.collective_compute(
                    "AllGather",
                    Alu.bypass,
                    replica_groups=RG,
                    ins=[l1slab[:].opt()],
                    outs=[table1[0:TROWS, :].opt()],
                )

                def edge_layer(table, ROWB, FH, layer):
                    scol, dcol = ROWB - 2, ROWB - 1
                    G = gpool.tile([P, NT * W, ROWB], dt.float32, tag=f"G{layer}")
                    nc.gpsimd.indirect_dma_start(
                        out=G[:],
                        out_offset=None,
                        in_=table[:],
                        in_offset=bass.IndirectOffsetOnAxis(ap=idx_s[:], axis=0),
                    )
                    G4 = G[:].rearrange("p (t w) f -> p t w f", w=W)
                    # z = s[src] + d[dst(self)]
                    z = wpool.tile([P, NT, W], dt.float32, tag=f"z{layer}")
                    nc.vector.tensor_tensor(
                        out=z[:],
                        in0=G4[:, :, :, scol : scol + 1].squeeze(),
                        in1=G4[:, :, 0:1, dcol : dcol + 1]
                        .rearrange("p t a b -> p t (a b)")
                        .to_broadcast([P, NT, W]),
                        op=Alu.add,
                    )
                    # e = max(z, 0.2 z)  (leaky relu)
                    e = wpool.tile([P, NT, W], dt.float32, tag=f"e{layer}")
                    nc.vector.scalar_tensor_tensor(
                        out=e[:], in0=z[:], scalar=NEG, in1=z[:],
                        op0=Alu.mult, op1=Alu.max,
                    )
                    # ex = exp(e); den = per-tile sum
                    ex = wpool.tile([P, NT, W], dt.float32, tag=f"ex{layer}")
                    nc.scalar.activation(ex[:], e[:], Act.Exp)
                    den = wpool.tile([P, NT], dt.float32, tag=f"den{layer}")
                    nc.vector.tensor_reduce(
                        den[:], ex[:], axis=mybir.AxisListType.X, op=Alu.add
                    )
                    rec = wpool.tile([P, NT], dt.float32, tag=f"rec{layer}")
                    nc.vector.reciprocal(rec[:], den[:])
                    # num[p,t,f] = sum_w ex[p,t,w] * G[p,t,w,f]
                    if layer == 1:
                        exb = wpool.tile([P, NT, W], dt.bfloat16, tag="exb")
                        nc.vector.tensor_copy(exb[:], ex[:])
                        hv = G.bitcast(dt.bfloat16)[:, :, 0:H].rearrange(
                            "p (t w) f -> p t f w", w=W
                        )
                        tmp = wpool.tile([P, NT, H, W], dt.bfloat16, tag="tmp1")
                        nc.vector.tensor_tensor(
                            out=tmp[:],
                            in0=hv,
                            in1=exb[:].unsqueeze(2).to_broadcast([P, NT, H, W]),
                            op=Alu.mult,
                        )
                        num = wpool.tile([P, NT, H], dt.float32, tag="num1")
                        nc.vector.tensor_reduce(
                            num[:], tmp[:], axis=mybir.AxisListType.X, op=Alu.add
                        )
                        o1 = wpool.tile([P, NT, H], dt.float32, tag="o1")
                        nc.vector.tensor_tensor(
                            out=o1[:],
                            in0=num[:],
                            in1=rec[:].unsqueeze(2).to_broadcast([P, NT, H]),
                            op=Alu.mult,
                        )
                        # h2aug = o1 @ W2aug + [b2|0]  (per-partition matvec)
                        tmp2 = wpool.tile([P, NT, 4, H], dt.float32, tag="tmp2")
                        nc.vector.tensor_tensor(
                            out=tmp2[:],
                            in0=o1[:].unsqueeze(2).to_broadcast([P, NT, 4, H]),
                            in1=w2b_s[:].unsqueeze(1).to_broadcast([P, NT, 4, H]),
                            op=Alu.mult,
                        )
                        h2t = wpool.tile([P, NT, 4], dt.float32, tag="h2t")
                        nc.vector.tensor_reduce(
                            h2t[:], tmp2[:], axis=mybir.AxisListType.X, op=Alu.add
                        )
                        h2b = wpool.tile([P, NT, 4], dt.float32, tag="h2b")
                        nc.vector.tensor_tensor(
                            out=h2b[:],
                            in0=h2t[:],
                            in1=b2a_s[:].unsqueeze(1).to_broadcast([P, NT, 4]),
                            op=Alu.add,
                        )
                        nc.sync.dma_start(
                            l2slab[0:NSH, :].rearrange("(t p) f -> p t f", p=P),
                            h2b[:],
                        )
                        nc.sync.dma_start(l2slab[NSH : NSH + 1, :], sent2[:])
                    else:
                        ov = G[:, :, 0:C].rearrange("p (t w) c -> p t c w", w=W)
                        tmp3 = wpool.tile([P, NT, C, W], dt.float32, tag="tmp3")
                        nc.vector.tensor_tensor(
                            out=tmp3[:],
                            in0=ov,
                            in1=ex[:].unsqueeze(2).to_broadcast([P, NT, C, W]),
                            op=Alu.mult,
                        )
                        num2 = wpool.tile([P, NT, C], dt.float32, tag="num2")
                        nc.vector.tensor_reduce(
                            num2[:], tmp3[:], axis=mybir.AxisListType.X, op=Alu.add
                        )
                        allout = wpool.tile([P, NT, C], dt.float32, tag="allout")
                        nc.vector.tensor_tensor(
                            out=allout[:],
                            in0=num2[:],
                            in1=rec[:].unsqueeze(2).to_broadcast([P, NT, C]),
                            op=Alu.mult,
                        )
                        nc.sync.dma_start(
                            out_p[:].rearrange("(t p) c -> p t c", p=P), allout[:]
                        )

                # ---- Phase C: layer-1 edge aggregation + h2 slab
                if phase == "ag1":
                    nc.sync.dma_start(out_p[:], table1[0:NSH, 0:C])
                    return
                edge_layer(table1, ROW1, H, 1)
                if phase == "gat1":
                    nc.sync.dma_start(out_p[:], l2slab[0:NSH, 0:C])
                    return
                nc.gpsimd.collective_compute(
                    "AllGather",
                    Alu.bypass,
                    replica_groups=RG,
                    ins=[l2slab[:].opt()],
                    outs=[table2[0:TROWS, :].opt()],
                )
                if phase == "ag2":
                    nc.sync.dma_start(out_p[:], table2[0:NSH, 0:C])
                    return
                # ---- Phase D: layer-2 edge aggregation -> unnormalized out
                edge_layer(table2, ROW2, C, 2)

            _emit()
    nc.compile()
    _patch_serialization(nc)
    return nc


def _prep(x, edge_index, W1, a_src1, a_dst1, b1, W2, a_src2, a_dst2, b2):
    ei = np.asarray(edge_index).astype(np.int64)
    src_all, dst_all = ei[0], ei[1]
    E = src_all.shape[0]
    counts = np.bincount(dst_all, minlength=N)
    W = int(counts.max()) + 1

    # table row position of node g: NSHE*(g//NSH) + g%NSH; sentinel at SENTROW
    g = np.arange(N, dtype=np.int64)
    pos = NSHE * (g // NSH) + (g % NSH)

    perm_e = np.argsort(dst_all, kind="stable")
    dsorted = dst_all[perm_e]
    ssorted = src_all[perm_e]
    starts = np.zeros(N + 1, np.int64)
    np.cumsum(counts, out=starts[1:])
    rank = np.arange(E, dtype=np.int64) - starts[dsorted]

    mat = np.full((N, W), SENTROW, np.int64)
    mat[:, 0] = pos
    mat[dsorted, 1 + rank] = pos[ssorted]

    idx_maps = []
    for c in range(NCORES):
        m = mat[c * NSH : (c + 1) * NSH].reshape(NT, P, W)
        idx_maps.append(
            np.ascontiguousarray(m.transpose(1, 0, 2).reshape(P, NT * W)).astype(
                np.int32
            )
        )

    bf = ml_dtypes.bfloat16
    W1aug = np.concatenate(
        [W1, (W1 @ a_src1)[:, None], (W1 @ a_dst1)[:, None]], axis=1
    ).astype(np.float32)
    w1s = (
        W1aug.reshape(KCH, P, AUG1).transpose(1, 0, 2).reshape(P, KCH * AUG1)
    ).astype(bf)
    v1 = np.concatenate([b1.astype(np.float32), np.zeros(2, np.float32)]).reshape(
        AUG1, 1
    )
    W2aug = np.concatenate(
        [W2, (W2 @ a_src2)[:, None], (W2 @ a_dst2)[:, None]], axis=1
    ).astype(np.float32)
    w2b = np.tile(W2aug.T.reshape(1, 4 * H), (P, 1)).astype(np.float32)
    b2a = np.tile(
        np.array([b2[0], b2[1], 0.0, 0.0], np.float32), (P, 1)
    ).astype(np.float32)

    xb = np.asarray(x, np.float32).astype(bf)
    in_maps = []
    for c in range(NCORES):
        in_maps.append(
            {
                "xs": np.ascontiguousarray(xb[NSH * c : NSH * (c + 1)].T),
                "w1s": w1s,
                "v1": v1,
                "idx": idx_maps[c],
                "w2b": w2b,
                "b2a": b2a,
            }
        )
    return W, in_maps


_NC_CACHE = {}


def _get_nc(W):
    if W not in _NC_CACHE:
        _NC_CACHE[W] = _build(W)
    return _NC_CACHE[W]


def kernel(**inputs):
    from concourse.bass_utils import run_bass_kernel_spmd

    W, in_maps = _prep(
        inputs["x"], inputs["edge_index"], inputs["W1"], inputs["a_src1"],
        inputs["a_dst1"], inputs["b1"], inputs["W2"], inputs["a_src2"],
        inputs["a_dst2"], inputs["b2"],
    )
    nc = _get_nc(W)
    res = run_bass_kernel_spmd(nc, in_maps, list(range(NCORES)))
    out = np.concatenate([res.results[c]["out"] for c in range(NCORES)], axis=0)
    out = out.astype(np.float64)
    mn, mx = out.min(), out.max()
    return (2.0 * (out - mn) / (mx - mn) - 1.0).astype(np.float32)


# revision 9
# speedup vs baseline: 2.5242x; 1.3274x over previous
"""Two-layer GAT (nn_GAT_82334523064895) on 8 TRN2 NeuronCores via Bass.

Strategy (8-way contiguous node sharding, SPMD single NEFF):
  1. x is transposed + bf16-cast on host so phase A streams it at line rate
     (no DMA-transpose); loads are spread over 4 DMA queues (SP/ACT/DVE/
     Pool) to overlap with the PE GEMM: hT = W1aug.T @ xT accumulated in
     PSUM over 64 k-chunks, W1aug = [W1 | W1@a_s1 | W1@a_d1].
  2. h rows packed as 36B bf16 table rows [h0..h15 | s | d] into a DRAM
     slab; small AllGather (295 KB) replicates the table.
  3. Edge phase as uniform-width padded ELL (W = 1 + global max in-degree):
     ONE indirect-DMA gather per layer fetches all rows (flat byte-offset
     indices precomputed on host; pad slots hit a sentinel row with
     s = -3e4 so exp -> 0). Softmax without max-shift; the weighted-sum
     multiply+reduce is split across DVE and GpSimd engines.
  4. Layer 2 identically with 8B bf16 rows [o2c0 | o2c1 | s2 | d2]
     (h2aug = o1 @ W2aug on DVE; b2 folded in pre-aggregation, valid
     since sum(alpha) = 1).
  5. Unnormalized outputs returned; global min/max rescale done on host.
Host does index prep (shared structure for both layers), x transpose/cast,
final rescale.
"""

import numpy as np
import ml_dtypes

N = 8192
F = 8192
H = 16
C = 2
NCORES = 8
NSH = N // NCORES          # nodes per core
P = 128
NT = NSH // P              # dst tiles per core
AUG1 = H + 2               # h .. s, d
KCH = F // P               # k chunks
ROW1 = AUG1                # bf16 elems per L1 table row (36B)
ROW2 = 4                   # bf16 elems per L2 table row (8B)
NSHE = NSH + 1             # slab rows: NSH nodes + 1 sentinel row
TROWS = NCORES * NSHE
SENTROW = NSH              # sentinel position = core-0 slab row NSH
NEG = 0.2
SENT = -30000.0


def _install_tilefix():
    """Split the Tile kernel-tail drain's sem waits across multiple drains
    (this walrus build rejects >1 sync wait on a CTRL instruction)."""
    import bass_rust
    from bass_rust import ScopedClock
    import concourse.tile as tile

    def _split_drain_and_barrier(self, tick_clock, wait_clock):
        nc = self.nc
        drain_inst = nc.sync.drain()
        wait_clock.add_sem_waits(
            drain_inst.ins, ScopedClock({None: tick_clock.global_clock})
        )
        si = drain_inst.ins.sync_info
        waits = list(si.on_wait) if si is not None else []
        if len(waits) > 1:
            si.on_wait = waits[:1]
            for i in range(1, len(waits)):
                d2 = nc.sync.drain()
                si2 = d2.ins.sync_info
                if si2 is None:
                    d2.ins.sync_info = bass_rust.SyncInfo(on_wait=[], on_update=[])
                    si2 = d2.ins.sync_info
                si2.on_wait = waits[i : i + 1]
        nc.all_engine_barrier()
        popped = nc._tile_sem_poison_stack.pop()
        assert popped is self._sem_poison
        nc.clear_and_free_semaphores(list(self.sems.allocated().values()))
        nc.all_engine_barrier()

    tile.TileContext._drain_and_barrier = _split_drain_and_barrier


def _split_multiwaits(d):
    """Walrus in this build accepts a single sync wait per instruction; hoist
    extra waits onto wait-only EventSemaphore carriers inserted just before."""
    n = 0
    for fn in d["functions"]:
        for blk in fn["blocks"]:
            newl = []
            for ins in blk["instructions"]:
                si = ins.get("sync_info")
                waits = (si or {}).get("on_wait") or []
                if len(waits) > 1:
                    for w in waits[:-1]:
                        n += 1
                        newl.append(
                            {
                                "debug": ins.get("debug"),
                                "engine": ins["engine"],
                                "ins": [],
                                "outs": [],
                                "name": f"{ins['name']}-ws{n}",
                                "opcode": "EventSemaphore",
                                "sync_info": {"on_update": [], "on_wait": [w]},
                            }
                        )
                    si["on_wait"] = [waits[-1]]
                newl.append(ins)
            blk["instructions"] = newl
    return d


def _patch_serialization(nc):
    import types
    import json

    orig = nc.to_json_bytes

    def to_json_bytes_patched(self):
        d = json.loads(orig())
        _split_multiwaits(d)
        return json.dumps(d).encode()

    nc.to_json_bytes = types.MethodType(to_json_bytes_patched, nc)


def _build(W, phase="full"):
    import concourse.bass as bass
    import concourse.bacc as bacc
    import concourse.mybir as mybir
    import concourse.tile as tile
    from concourse.masks import make_identity

    _install_tilefix()
    dt = mybir.dt
    Alu = mybir.AluOpType
    Act = mybir.ActivationFunctionType
    RG = [list(range(NCORES))]
    NW = NT * W
    HH = H // 2

    nc = bacc.Bacc("TRN2", debug=False)
    xs_p = nc.declare_dram_parameter("xs", [F, NSH], dt.bfloat16, isOutput=False)
    w1_p = nc.declare_dram_parameter("w1s", [P, KCH * AUG1], dt.bfloat16, isOutput=False)
    v1_p = nc.declare_dram_parameter("v1", [AUG1, 1], dt.float32, isOutput=False)
    idx1_p = nc.declare_dram_parameter("idx1", [P, NW], dt.int32, isOutput=False)
    idx2_p = nc.declare_dram_parameter("idx2", [P, NW], dt.int32, isOutput=False)
    w2b_p = nc.declare_dram_parameter("w2b", [P, 4 * H], dt.float32, isOutput=False)
    b2a_p = nc.declare_dram_parameter("b2a", [P, 4], dt.float32, isOutput=False)
    out_p = nc.declare_dram_parameter("out", [NSH, C], dt.float32, isOutput=True)

    with tile.TileContext(nc) as tc:
        with (
            tc.tile_pool(name="const", bufs=1) as cpool,
            tc.tile_pool(name="xload", bufs=6) as xpool,
            tc.tile_pool(name="work", bufs=1) as wpool,
            tc.tile_pool(name="gath", bufs=1) as gpool,
            tc.tile_pool(name="pst", bufs=2, space="PSUM") as ppool,
            tc.tile_pool(name="psacc", bufs=1, space="PSUM") as psacc,
            tc.tile_pool(name="dram", bufs=1, space="DRAM") as dpool,
        ):
            def _emit():
                # ---- constants
                w1_s = cpool.tile([P, KCH * AUG1], dt.bfloat16)
                nc.sync.dma_start(w1_s[:], w1_p[:])
                w1v = w1_s[:].rearrange("p (c f) -> p c f", f=AUG1)
                v1_s = cpool.tile([AUG1, 1], dt.float32)
                nc.sync.dma_start(v1_s[:], v1_p[:])
                idx1_s = cpool.tile([P, NW], dt.int32)
                nc.sync.dma_start(idx1_s[:], idx1_p[:])
                idx2_s = cpool.tile([P, NW], dt.int32)
                nc.sync.dma_start(idx2_s[:], idx2_p[:])
                w2b_s = cpool.tile([P, 4, H], dt.float32)
                nc.sync.dma_start(w2b_s[:], w2b_p[:].rearrange("p (c k) -> p c k", k=H))
                b2a_s = cpool.tile([P, 4], dt.float32)
                nc.sync.dma_start(b2a_s[:], b2a_p[:])
                ident = cpool.tile([P, P], dt.float32)
                make_identity(nc, ident[:])
                sent1 = cpool.tile([1, ROW1], dt.bfloat16)
                nc.gpsimd.memset(sent1[:], SENT)
                sent2 = cpool.tile([1, ROW2], dt.bfloat16)
                nc.gpsimd.memset(sent2[:], SENT)

                # ---- internal DRAM
                l1slab = dpool.tile([NSHE, ROW1], dt.bfloat16)
                table1 = dpool.tile([TROWS, ROW1], dt.bfloat16, addr_space="Shared")
                l2slab = dpool.tile([NSHE, ROW2], dt.bfloat16)
                table2 = dpool.tile([TROWS, ROW2], dt.bfloat16, addr_space="Shared")

                # ---- Phase A: hT = W1aug.T @ xT accumulated over k chunks;
                # x loads spread across 4 DMA queues to overlap with PE.
                qs = [nc.sync, nc.scalar, nc.gpsimd]
                hps0 = psacc.tile([AUG1, 512], dt.float32, tag="hps0")
                hps1 = psacc.tile([AUG1, 512], dt.float32, tag="hps1")
                hps = [hps0, hps1]
                for ck in range(KCH // 2):
                    xt = xpool.tile([P, 2, NSH], dt.bfloat16, tag="xt")
                    qs[ck % 3].dma_start(
                        xt[:],
                        xs_p[ck * 2 * P : (ck + 1) * 2 * P, :].rearrange(
                            "(j p) n -> p j n", p=P
                        ),
                    )
                    for j in range(2):
                        c2 = 2 * ck + j
                        for mh in range(2):
                            nc.tensor.matmul(
                                hps[mh][:],
                                w1v[:, c2, :],
                                xt[:, j, mh * 512 : (mh + 1) * 512],
                                start=(c2 == 0),
                                stop=(c2 == KCH - 1),
                            )
                hT = cpool.tile([AUG1, NSH], dt.float32)
                for mh in range(2):
                    nc.scalar.activation(
                        hT[:, mh * 512 : (mh + 1) * 512],
                        hps[mh][:],
                        Act.Identity,
                        bias=v1_s[:],
                    )

                # ---- Phase B: pack h rows (all bf16) -> slab -> AllGather
                slabsb = cpool.tile([P, NT, ROW1], dt.bfloat16)
                for t in range(NT):
                    hr_ps = ppool.tile([P, AUG1], dt.float32, tag="hrps")
                    nc.tensor.transpose(
                        hr_ps[:], hT[:, t * P : (t + 1) * P], ident[:AUG1, :AUG1]
                    )
                    nc.vector.tensor_copy(slabsb[:, t, 0:H], hr_ps[:, 0:H])
                    nc.vector.tensor_copy(slabsb[:, t, H:AUG1], hr_ps[:, H:AUG1])
                nc.sync.dma_start(
                    l1slab[0:NSH, :].rearrange("(t p) f -> p t f", p=P), slabsb[:]
                )
                nc.sync.dma_start(l1slab[NSH : NSH + 1, :], sent1[:])
                if phase == "gemm":
                    nc.sync.dma_start(out_p[:], l1slab[0:NSH, 0:C])
                    return
                nc.gpsimd.collective_compute(
                    "AllGather",
                    Alu.bypass,
                    replica_groups=RG,
                    ins=[l1slab[:].opt()],
                    outs=[table1[0:TROWS, :].opt()],
                )

                def edge_layer(table, idx_s, ROWB, layer):
                    scol, dcol = ROWB - 2, ROWB - 1
                    G = gpool.tile([P, NW * ROWB], dt.bfloat16, tag=f"G{layer}")
                    nc.gpsimd.indirect_dma_start(
                        out=G[:],
                        out_offset=None,
                        in_=table[:].rearrange("r f -> (r f)").unsqueeze(0),
                        in_offset=bass.IndirectOffsetOnAxis(ap=idx_s[:], axis=1),
                    )
                    G4 = G[:].rearrange("p (t w f) -> p t w f", w=W, f=ROWB)
                    # z = s[src] + d[dst(self)]
                    z = wpool.tile([P, NT, W], dt.float32, tag=f"z{layer}")
                    nc.vector.tensor_tensor(
                        out=z[:],
                        in0=G4[:, :, :, scol : scol + 1].squeeze(),
                        in1=G4[:, :, 0:1, dcol : dcol + 1]
                        .rearrange("p t a b -> p t (a b)")
                        .to_broadcast([P, NT, W]),
                        op=Alu.add,
                    )
                    # e = max(z, 0.2 z)  (leaky relu)
                    e = wpool.tile([P, NT, W], dt.float32, tag=f"e{layer}")
                    nc.vector.scalar_tensor_tensor(
                        out=e[:], in0=z[:], scalar=NEG, in1=z[:],
                        op0=Alu.mult, op1=Alu.max,
                    )
                    # ex = exp(e) (bf16); den = per-tile sum (f32)
                    exb = wpool.tile([P, NT, W], dt.bfloat16, tag=f"exb{layer}")
                    nc.scalar.activation(exb[:], e[:], Act.Exp)
                    den = wpool.tile([P, NT], dt.float32, tag=f"den{layer}")
                    nc.vector.tensor_reduce(
                        den[:], exb[:], axis=mybir.AxisListType.X, op=Alu.add
                    )
                    rec = wpool.tile([P, NT], dt.float32, tag=f"rec{layer}")
                    nc.vector.reciprocal(rec[:], den[:])
                    return G4, exb, rec

                # ---- Phase C: layer-1 edge aggregation + h2 slab
                if phase == "ag1":
                    nc.sync.dma_start(out_p[:], table1[0:NSH, 0:C])
                    return
                G4, exb, rec = edge_layer(table1, idx1_s, ROW1, 1)
                # num[p,t,f] = sum_w exb[p,t,w] * h[p,t,w,f]; split DVE/Pool
                hv = G4[:, :, :, 0:H].rearrange("p t w f -> p t f w")
                num = wpool.tile([P, NT, H], dt.float32, tag="num1")
                FA = 4  # DVE multiplies features [0, FA); Pool does the rest
                tmpA = wpool.tile([P, NT, FA, W], dt.bfloat16, tag="tmpA")
                tmpB = wpool.tile([P, NT, H - FA, W], dt.bfloat16, tag="tmpB")
                for eng, tmp, f0, fn in (
                    (nc.vector, tmpA, 0, FA),
                    (nc.gpsimd, tmpB, FA, H),
                ):
                    eng.tensor_tensor(
                        out=tmp[:],
                        in0=hv[:, :, f0:fn, :],
                        in1=exb[:].unsqueeze(2).to_broadcast([P, NT, fn - f0, W]),
                        op=Alu.mult,
                    )
                for tmp, f0, fn in ((tmpA, 0, FA), (tmpB, FA, H)):
                    nc.vector.tensor_reduce(
                        num[:, :, f0:fn], tmp[:],
                        axis=mybir.AxisListType.X, op=Alu.add,
                    )
                o1 = wpool.tile([P, NT, H], dt.float32, tag="o1")
                nc.vector.tensor_tensor(
                    out=o1[:],
                    in0=num[:],
                    in1=rec[:].unsqueeze(2).to_broadcast([P, NT, H]),
                    op=Alu.mult,
                )
                # h2aug = o1 @ W2aug + [b2|0]  (per-partition matvec)
                tmp2 = wpool.tile([P, NT, 4, H], dt.float32, tag="tmp2")
                nc.vector.tensor_tensor(
                    out=tmp2[:],
                    in0=o1[:].unsqueeze(2).to_broadcast([P, NT, 4, H]),
                    in1=w2b_s[:].unsqueeze(1).to_broadcast([P, NT, 4, H]),
                    op=Alu.mult,
                )
                h2t = wpool.tile([P, NT, 4], dt.float32, tag="h2t")
                nc.vector.tensor_reduce(
                    h2t[:], tmp2[:], axis=mybir.AxisListType.X, op=Alu.add
                )
                h2b = wpool.tile([P, NT, 4], dt.bfloat16, tag="h2b")
                nc.vector.tensor_tensor(
                    out=h2b[:],
                    in0=h2t[:],
                    in1=b2a_s[:].unsqueeze(1).to_broadcast([P, NT, 4]),
                    op=Alu.add,
                )
                nc.sync.dma_start(
                    l2slab[0:NSH, :].rearrange("(t p) f -> p t f", p=P), h2b[:]
                )
                nc.sync.dma_start(l2slab[NSH : NSH + 1, :], sent2[:])
                if phase == "gat1":
                    nc.sync.dma_start(out_p[:], l2slab[0:NSH, 0:C])
                    return
                nc.gpsimd.collective_compute(
                    "AllGather",
                    Alu.bypass,
                    replica_groups=RG,
                    ins=[l2slab[:].opt()],
                    outs=[table2[0:TROWS, :].opt()],
                )
                if phase == "ag2":
                    nc.sync.dma_start(out_p[:], table2[0:NSH, 0:C])
                    return
                # ---- Phase D: layer-2 edge aggregation -> unnormalized out
                G24, exb2, rec2 = edge_layer(table2, idx2_s, ROW2, 2)
                ov = G24[:, :, :, 0:C].rearrange("p t w c -> p t c w")
                tmp3 = wpool.tile([P, NT, C, W], dt.bfloat16, tag="tmp3")
                nc.vector.tensor_tensor(
                    out=tmp3[:],
                    in0=ov,
                    in1=exb2[:].unsqueeze(2).to_broadcast([P, NT, C, W]),
                    op=Alu.mult,
                )
                num2 = wpool.tile([P, NT, C], dt.float32, tag="num2")
                nc.vector.tensor_reduce(
                    num2[:], tmp3[:], axis=mybir.AxisListType.X, op=Alu.add
                )
                allout = wpool.tile([P, NT, C], dt.float32, tag="allout")
                nc.vector.tensor_tensor(
                    out=allout[:],
                    in0=num2[:],
                    in1=rec2[:].unsqueeze(2).to_broadcast([P, NT, C]),
                    op=Alu.mult,
                )
                nc.sync.dma_start(
                    out_p[:].rearrange("(t p) c -> p t c", p=P), allout[:]
                )

            _emit()
    nc.compile()
    _patch_serialization(nc)
    return nc


def _prep(x, edge_index, W1, a_src1, a_dst1, b1, W2, a_src2, a_dst2, b2):
    ei = np.asarray(edge_index).astype(np.int64)
    src_all, dst_all = ei[0], ei[1]
    E = src_all.shape[0]
    counts = np.bincount(dst_all, minlength=N)
    W = int(counts.max()) + 1

    # table row position of node g: NSHE*(g//NSH) + g%NSH; sentinel at SENTROW
    g = np.arange(N, dtype=np.int64)
    pos = NSHE * (g // NSH) + (g % NSH)

    perm_e = np.argsort(dst_all, kind="stable")
    dsorted = dst_all[perm_e]
    ssorted = src_all[perm_e]
    starts = np.zeros(N + 1, np.int64)
    np.cumsum(counts, out=starts[1:])
    rank = np.arange(E, dtype=np.int64) - starts[dsorted]

    mat = np.full((N, W), SENTROW, np.int64)
    mat[:, 0] = pos
    mat[dsorted, 1 + rank] = pos[ssorted]

    idx1_maps, idx2_maps = [], []
    for c in range(NCORES):
        m = mat[c * NSH : (c + 1) * NSH].reshape(NT, P, W)
        m = np.ascontiguousarray(m.transpose(1, 0, 2).reshape(P, NT * W))
        idx1_maps.append((m * ROW1).astype(np.int32))
        idx2_maps.append((m * ROW2).astype(np.int32))

    bf = ml_dtypes.bfloat16
    W1aug = np.concatenate(
        [W1, (W1 @ a_src1)[:, None], (W1 @ a_dst1)[:, None]], axis=1
    ).astype(np.float32)
    w1s = (
        W1aug.reshape(KCH, P, AUG1).transpose(1, 0, 2).reshape(P, KCH * AUG1)
    ).astype(bf)
    v1 = np.concatenate([b1.astype(np.float32), np.zeros(2, np.float32)]).reshape(
        AUG1, 1
    )
    W2aug = np.concatenate(
        [W2, (W2 @ a_src2)[:, None], (W2 @ a_dst2)[:, None]], axis=1
    ).astype(np.float32)
    w2b = np.tile(W2aug.T.reshape(1, 4 * H), (P, 1)).astype(np.float32)
    b2a = np.tile(
        np.array([b2[0], b2[1], 0.0, 0.0], np.float32), (P, 1)
    ).astype(np.float32)

    xb = np.asarray(x, np.float32).astype(bf)
    in_maps = []
    for c in range(NCORES):
        in_maps.append(
            {
                "xs": np.ascontiguousarray(xb[NSH * c : NSH * (c + 1)].T),
                "w1s": w1s,
                "v1": v1,
                "idx1": idx1_maps[c],
                "idx2": idx2_maps[c],
                "w2b": w2b,
                "b2a": b2a,
            }
        )
    return W, in_maps


_NC_CACHE = {}


def _get_nc(W):
    if W not in _NC_CACHE:
        _NC_CACHE[W] = _build(W)
    return _NC_CACHE[W]


def kernel(**inputs):
    from concourse.bass_utils import run_bass_kernel_spmd

    W, in_maps = _prep(
        inputs["x"], inputs["edge_index"], inputs["W1"], inputs["a_src1"],
        inputs["a_dst1"], inputs["b1"], inputs["W2"], inputs["a_src2"],
        inputs["a_dst2"], inputs["b2"],
    )
    nc = _get_nc(W)
    res = run_bass_kernel_spmd(nc, in_maps, list(range(NCORES)))
    out = np.concatenate([res.results[c]["out"] for c in range(NCORES)], axis=0)
    out = out.astype(np.float64)
    mn, mx = out.min(), out.max()
    return (2.0 * (out - mn) / (mx - mn) - 1.0).astype(np.float32)


# revision 18
# speedup vs baseline: 2.6491x; 1.0495x over previous
"""Two-layer GAT (nn_GAT_82334523064895) on 8 TRN2 NeuronCores via Bass.

Strategy (8-way contiguous node sharding, SPMD single NEFF):
  1. x is transposed + bf16-cast on host so phase A streams it at line rate
     (no DMA-transpose); loads are spread over 4 DMA queues (SP/ACT/DVE/
     Pool) to overlap with the PE GEMM: hT = W1aug.T @ xT accumulated in
     PSUM over 64 k-chunks, W1aug = [W1 | W1@a_s1 | W1@a_d1].
  2. h rows packed as 36B bf16 table rows [h0..h15 | s | d] into a DRAM
     slab; small AllGather (295 KB) replicates the table.
  3. Edge phase as uniform-width padded ELL (W = 1 + global max in-degree):
     ONE indirect-DMA gather per layer fetches all rows (flat byte-offset
     indices precomputed on host; pad slots hit a sentinel row with
     s = -3e4 so exp -> 0). Softmax without max-shift; the weighted-sum
     multiply+reduce is split across DVE and GpSimd engines.
  4. Layer 2 identically with 8B bf16 rows [o2c0 | o2c1 | s2 | d2]
     (h2aug = o1 @ W2aug on DVE; b2 folded in pre-aggregation, valid
     since sum(alpha) = 1).
  5. Unnormalized outputs returned; global min/max rescale done on host.
Host does index prep (shared structure for both layers), x transpose/cast,
final rescale.
"""

import numpy as np
import ml_dtypes

N = 8192
F = 8192
H = 16
C = 2
NCORES = 8
NSH = N // NCORES          # nodes per core
P = 128
NT = NSH // P              # dst tiles per core
AUG1 = H + 2               # h .. s, d
KCH = F // P               # k chunks
ROW1 = AUG1                # bf16 elems per L1 table row (36B)
ROW2 = 4                   # bf16 elems per L2 table row (8B)
NSHE = NSH + 1             # slab rows: NSH nodes + 1 sentinel row
TROWS = NCORES * NSHE
SENTROW = NSH              # sentinel position = core-0 slab row NSH
NEG = 0.2
SENT = -30000.0


def _install_tilefix():
    """Split the Tile kernel-tail drain's sem waits across multiple drains
    (this walrus build rejects >1 sync wait on a CTRL instruction)."""
    import bass_rust
    from bass_rust import ScopedClock
    import concourse.tile as tile

    def _split_drain_and_barrier(self, tick_clock, wait_clock):
        nc = self.nc
        drain_inst = nc.sync.drain()
        wait_clock.add_sem_waits(
            drain_inst.ins, ScopedClock({None: tick_clock.global_clock})
        )
        si = drain_inst.ins.sync_info
        waits = list(si.on_wait) if si is not None else []
        if len(waits) > 1:
            si.on_wait = waits[:1]
            for i in range(1, len(waits)):
                d2 = nc.sync.drain()
                si2 = d2.ins.sync_info
                if si2 is None:
                    d2.ins.sync_info = bass_rust.SyncInfo(on_wait=[], on_update=[])
                    si2 = d2.ins.sync_info
                si2.on_wait = waits[i : i + 1]
        nc.all_engine_barrier()
        popped = nc._tile_sem_poison_stack.pop()
        assert popped is self._sem_poison
        nc.clear_and_free_semaphores(list(self.sems.allocated().values()))
        nc.all_engine_barrier()

    tile.TileContext._drain_and_barrier = _split_drain_and_barrier


def _split_multiwaits(d):
    """Walrus in this build accepts a single sync wait per instruction; hoist
    extra waits onto wait-only EventSemaphore carriers inserted just before."""
    n = 0
    for fn in d["functions"]:
        for blk in fn["blocks"]:
            newl = []
            for ins in blk["instructions"]:
                si = ins.get("sync_info")
                waits = (si or {}).get("on_wait") or []
                if len(waits) > 1:
                    for w in waits[:-1]:
                        n += 1
                        newl.append(
                            {
                                "debug": ins.get("debug"),
                                "engine": ins["engine"],
                                "ins": [],
                                "outs": [],
                                "name": f"{ins['name']}-ws{n}",
                                "opcode": "EventSemaphore",
                                "sync_info": {"on_update": [], "on_wait": [w]},
                            }
                        )
                    si["on_wait"] = [waits[-1]]
                newl.append(ins)
            blk["instructions"] = newl
    return d


def _patch_serialization(nc):
    import types
    import json

    orig = nc.to_json_bytes

    def to_json_bytes_patched(self):
        d = json.loads(orig())
        _split_multiwaits(d)
        return json.dumps(d).encode()

    nc.to_json_bytes = types.MethodType(to_json_bytes_patched, nc)


def _build(W, phase="full"):
    import concourse.bass as bass
    import concourse.bacc as bacc
    import concourse.mybir as mybir
    import concourse.tile as tile
    from concourse.masks import make_identity

    _install_tilefix()
    dt = mybir.dt
    Alu = mybir.AluOpType
    Act = mybir.ActivationFunctionType
    RG = [list(range(NCORES))]
    NW = NT * W
    HH = H // 2

    nc = bacc.Bacc("TRN2", debug=False)
    xs_p = nc.declare_dram_parameter("xs", [F, NSH], dt.bfloat16, isOutput=False)
    w1_p = nc.declare_dram_parameter("w1s", [P, KCH * AUG1], dt.bfloat16, isOutput=False)
    v1_p = nc.declare_dram_parameter("v1", [AUG1, 1], dt.float32, isOutput=False)
    idx1_p = nc.declare_dram_parameter("idx1", [P, NW], dt.int32, isOutput=False)
    idx2_p = nc.declare_dram_parameter("idx2", [P, NW], dt.int32, isOutput=False)
    w2b_p = nc.declare_dram_parameter("w2b", [P, 4 * H], dt.float32, isOutput=False)
    b2a_p = nc.declare_dram_parameter("b2a", [P, 4], dt.float32, isOutput=False)
    out_p = nc.declare_dram_parameter("out", [NSH, C], dt.float32, isOutput=True)

    with tile.TileContext(nc) as tc:
        with (
            tc.tile_pool(name="const", bufs=1) as cpool,
            tc.tile_pool(name="xload", bufs=6) as xpool,
            tc.tile_pool(name="work", bufs=1) as wpool,
            tc.tile_pool(name="gath", bufs=1) as gpool,
            tc.tile_pool(name="pst", bufs=2, space="PSUM") as ppool,
            tc.tile_pool(name="psacc", bufs=1, space="PSUM") as psacc,
            tc.tile_pool(name="dram", bufs=1, space="DRAM") as dpool,
        ):
            def _emit():
                # ---- constants needed before/during phase A (SP queue head)
                w1_s = cpool.tile([P, KCH * AUG1], dt.bfloat16)
                nc.sync.dma_start(w1_s[:], w1_p[:])
                w1v = w1_s[:].rearrange("p (c f) -> p c f", f=AUG1)
                v1_s = cpool.tile([AUG1, 1], dt.float32)
                nc.sync.dma_start(v1_s[:], v1_p[:])
                ident = cpool.tile([P, P], dt.float32)
                make_identity(nc, ident[:])
                sent1 = cpool.tile([1, ROW1], dt.bfloat16)
                nc.gpsimd.memset(sent1[:], SENT)
                sent2 = cpool.tile([1, ROW2], dt.bfloat16)
                nc.gpsimd.memset(sent2[:], SENT)

                # ---- internal DRAM
                l1slab = dpool.tile([NSHE, ROW1], dt.bfloat16)
                table1 = dpool.tile([TROWS, ROW1], dt.bfloat16, addr_space="Shared")
                l2slab = dpool.tile([NSHE, ROW2], dt.bfloat16)
                table2 = dpool.tile([TROWS, ROW2], dt.bfloat16, addr_space="Shared")

                # ---- Phase A: hT = W1aug.T @ xT accumulated over k chunks;
                # x loads spread across 4 DMA queues to overlap with PE.
                qs = [nc.sync, nc.scalar, nc.gpsimd]
                hps0 = psacc.tile([AUG1, 512], dt.float32, tag="hps0")
                hps1 = psacc.tile([AUG1, 512], dt.float32, tag="hps1")
                hps = [hps0, hps1]
                for ck in range(KCH // 2):
                    xt = xpool.tile([P, 2, NSH], dt.bfloat16, tag="xt")
                    qs[ck % 3].dma_start(
                        xt[:],
                        xs_p[ck * 2 * P : (ck + 1) * 2 * P, :].rearrange(
                            "(j p) n -> p j n", p=P
                        ),
                    )
                    for j in range(2):
                        c2 = 2 * ck + j
                        for mh in range(2):
                            nc.tensor.matmul(
                                hps[mh][:],
                                w1v[:, c2, :],
                                xt[:, j, mh * 512 : (mh + 1) * 512],
                                start=(c2 == 0),
                                stop=(c2 == KCH - 1),
                            )
                # constants only needed from the edge phase on (after x loads)
                idx1_s = cpool.tile([P, NW], dt.int32)
                nc.sync.dma_start(idx1_s[:], idx1_p[:])
                idx2_s = cpool.tile([P, NW], dt.int32)
                nc.sync.dma_start(idx2_s[:], idx2_p[:])
                w2b_s = cpool.tile([P, 4, H], dt.float32)
                nc.sync.dma_start(w2b_s[:], w2b_p[:].rearrange("p (c k) -> p c k", k=H))
                b2a_s = cpool.tile([P, 4], dt.float32)
                nc.sync.dma_start(b2a_s[:], b2a_p[:])

                hT = cpool.tile([AUG1, NSH], dt.float32)
                for mh in range(2):
                    nc.scalar.activation(
                        hT[:, mh * 512 : (mh + 1) * 512],
                        hps[mh][:],
                        Act.Identity,
                        bias=v1_s[:],
                    )

                # ---- Phase B: pack h rows (all bf16) -> slab -> AllGather
                slabsb = cpool.tile([P, NT, ROW1], dt.bfloat16)
                for t in range(NT):
                    hr_ps = ppool.tile([P, AUG1], dt.float32, tag="hrps")
                    nc.tensor.transpose(
                        hr_ps[:], hT[:, t * P : (t + 1) * P], ident[:AUG1, :AUG1]
                    )
                    nc.vector.tensor_copy(slabsb[:, t, :], hr_ps[:])
                nc.sync.dma_start(
                    l1slab[0:NSH, :].rearrange("(t p) f -> p t f", p=P), slabsb[:]
                )
                nc.sync.dma_start(l1slab[NSH : NSH + 1, :], sent1[:])

                def dump2(view):
                    st = wpool.tile([P, NT, C], dt.float32, tag="dump")
                    nc.vector.tensor_copy(st[:], view)
                    nc.sync.dma_start(
                        out_p[:].rearrange("(t p) c -> p t c", p=P), st[:]
                    )

                def dumptab(table, ROWB):
                    tt = wpool.tile([P, NT, ROWB], dt.bfloat16, tag="dumpt")
                    nc.sync.dma_start(
                        tt[:], table[0:NSH, :].rearrange("(t p) f -> p t f", p=P)
                    )
                    dump2(tt[:, :, 0:C])

                if phase == "gemm":
                    dump2(slabsb[:, :, 0:C])
                    return
                nc.gpsimd.collective_compute(
                    "AllGather",
                    Alu.bypass,
                    replica_groups=RG,
                    ins=[l1slab[:].opt()],
                    outs=[table1[0:TROWS, :].opt()],
                )

                def edge_layer(table, idx_s, ROWB, layer):
                    scol, dcol = ROWB - 2, ROWB - 1
                    G = gpool.tile([P, NW * ROWB], dt.bfloat16, tag=f"G{layer}")
                    nc.gpsimd.indirect_dma_start(
                        out=G[:],
                        out_offset=None,
                        in_=table[:].rearrange("r f -> (r f)").unsqueeze(0),
                        in_offset=bass.IndirectOffsetOnAxis(ap=idx_s[:], axis=1),
                    )
                    G4 = G[:].rearrange("p (t w f) -> p t w f", w=W, f=ROWB)
                    # z = s[src] + d[dst(self)]
                    z = wpool.tile([P, NT, W], dt.float32, tag=f"z{layer}")
                    nc.vector.tensor_tensor(
                        out=z[:],
                        in0=G4[:, :, :, scol : scol + 1].squeeze(),
                        in1=G4[:, :, 0:1, dcol : dcol + 1]
                        .rearrange("p t a b -> p t (a b)")
                        .to_broadcast([P, NT, W]),
                        op=Alu.add,
                    )
                    # e = max(z, 0.2 z)  (leaky relu); ex = exp(e) (bf16)
                    e = wpool.tile([P, NT, W], dt.float32, tag=f"e{layer}")
                    nc.vector.scalar_tensor_tensor(
                        out=e[:], in0=z[:], scalar=NEG, in1=z[:],
                        op0=Alu.mult, op1=Alu.max,
                    )
                    exb = wpool.tile([P, NT, W], dt.bfloat16, tag=f"exb{layer}")
                    nc.scalar.activation(exb[:], e[:], Act.Exp)
                    # den = per-tile sum (f32)
                    den = wpool.tile([P, NT], dt.float32, tag=f"den{layer}")
                    nc.vector.tensor_reduce(
                        den[:], exb[:], axis=mybir.AxisListType.X, op=Alu.add
                    )
                    rec = wpool.tile([P, NT], dt.float32, tag=f"rec{layer}")
                    nc.vector.reciprocal(rec[:], den[:])
                    return G4, exb, rec

                # ---- Phase C: layer-1 edge aggregation + h2 slab
                if phase == "ag1":
                    dumptab(table1, ROW1)
                    return
                G4, exb, rec = edge_layer(table1, idx1_s, ROW1, 1)
                if phase == "gonly":
                    dump2(G4[:, :, 0, 0:C])
                    return
                # num[p,t,f] = sum_w exb[p,t,w] * h[p,t,w,f].
                # Multiply on Pool in 4-feature chunks; reduce on DVE as a
                # bf16 pairwise tree (packed 2-byte APs run at DVE 2x rate)
                # with a final f32 tensor_reduce.
                hv = G4[:, :, :, 0:H].rearrange("p t w f -> p t f w")
                num = wpool.tile([P, NT, H], dt.float32, tag="num1")
                FC = 4
                tmp = wpool.tile([P, NT, H, W], dt.bfloat16, tag="tmp1")
                for f0 in range(0, H, FC):
                    nc.gpsimd.tensor_tensor(
                        out=tmp[:, :, f0 : f0 + FC, :],
                        in0=hv[:, :, f0 : f0 + FC, :],
                        in1=exb[:].unsqueeze(2).to_broadcast([P, NT, FC, W]),
                        op=Alu.mult,
                    )
                    src = tmp[:, :, f0 : f0 + FC, :]
                    w = W
                    while w > 8:
                        w //= 2
                        half = wpool.tile(
                            [P, NT, FC, w], dt.bfloat16, tag=f"tr{w}", bufs=2
                        )
                        nc.vector.tensor_tensor(
                            out=half[:],
                            in0=src[:, :, :, 0:w],
                            in1=src[:, :, :, w : 2 * w],
                            op=Alu.add,
                        )
                        src = half[:]
                    nc.vector.tensor_reduce(
                        num[:, :, f0 : f0 + FC], src,
                        axis=mybir.AxisListType.X, op=Alu.add,
                    )
                o1 = wpool.tile([P, NT, H], dt.float32, tag="o1")
                nc.vector.tensor_tensor(
                    out=o1[:],
                    in0=num[:],
                    in1=rec[:].unsqueeze(2).to_broadcast([P, NT, H]),
                    op=Alu.mult,
                )
                # h2aug = o1 @ W2aug + [b2|0]  (per-partition matvec)
                tmp2 = wpool.tile([P, NT, 4, H], dt.float32, tag="tmp2")
                nc.vector.tensor_tensor(
                    out=tmp2[:],
                    in0=o1[:].unsqueeze(2).to_broadcast([P, NT, 4, H]),
                    in1=w2b_s[:].unsqueeze(1).to_broadcast([P, NT, 4, H]),
                    op=Alu.mult,
                )
                h2t = wpool.tile([P, NT, 4], dt.float32, tag="h2t")
                nc.vector.tensor_reduce(
                    h2t[:], tmp2[:], axis=mybir.AxisListType.X, op=Alu.add
                )
                h2b = wpool.tile([P, NT, 4], dt.bfloat16, tag="h2b")
                nc.vector.tensor_tensor(
                    out=h2b[:],
                    in0=h2t[:],
                    in1=b2a_s[:].unsqueeze(1).to_broadcast([P, NT, 4]),
                    op=Alu.add,
                )
                nc.sync.dma_start(
                    l2slab[0:NSH, :].rearrange("(t p) f -> p t f", p=P), h2b[:]
                )
                nc.sync.dma_start(l2slab[NSH : NSH + 1, :], sent2[:])
                if phase == "gat1":
                    dump2(h2b[:, :, 0:C])
                    return
                nc.gpsimd.collective_compute(
                    "AllGather",
                    Alu.bypass,
                    replica_groups=RG,
                    ins=[l2slab[:].opt()],
                    outs=[table2[0:TROWS, :].opt()],
                )
                if phase == "ag2":
                    dumptab(table2, ROW2)
                    return
                # ---- Phase D: layer-2 edge aggregation -> unnormalized out
                G24, exb2, rec2 = edge_layer(table2, idx2_s, ROW2, 2)
                ov = G24[:, :, :, 0:C].rearrange("p t w c -> p t c w")
                tmp3 = wpool.tile([P, NT, C, W], dt.bfloat16, tag="tmp3")
                nc.gpsimd.tensor_tensor(
                    out=tmp3[:],
                    in0=ov,
                    in1=exb2[:].unsqueeze(2).to_broadcast([P, NT, C, W]),
                    op=Alu.mult,
                )
                num2 = wpool.tile([P, NT, C], dt.float32, tag="num2")
                nc.vector.tensor_reduce(
                    num2[:], tmp3[:], axis=mybir.AxisListType.X, op=Alu.add
                )
                allout = wpool.tile([P, NT, C], dt.float32, tag="allout")
                nc.vector.tensor_tensor(
                    out=allout[:],
                    in0=num2[:],
                    in1=rec2[:].unsqueeze(2).to_broadcast([P, NT, C]),
                    op=Alu.mult,
                )
                nc.sync.dma_start(
                    out_p[:].rearrange("(t p) c -> p t c", p=P), allout[:]
                )

            _emit()
    nc.compile()
    _patch_serialization(nc)
    return nc


def _prep(x, edge_index, W1, a_src1, a_dst1, b1, W2, a_src2, a_dst2, b2):
    ei = np.asarray(edge_index).astype(np.int64)
    src_all, dst_all = ei[0], ei[1]
    E = src_all.shape[0]
    counts = np.bincount(dst_all, minlength=N)
    W = 64  # pow2 ELL width (pairwise-tree friendly); widen if degree demands
    while W < int(counts.max()) + 1:
        W *= 2

    # table row position of node g: NSHE*(g//NSH) + g%NSH; sentinel at SENTROW
    g = np.arange(N, dtype=np.int64)
    pos = NSHE * (g // NSH) + (g % NSH)

    perm_e = np.argsort(dst_all, kind="stable")
    dsorted = dst_all[perm_e]
    ssorted = src_all[perm_e]
    starts = np.zeros(N + 1, np.int64)
    np.cumsum(counts, out=starts[1:])
    rank = np.arange(E, dtype=np.int64) - starts[dsorted]

    mat = np.full((N, W), SENTROW, np.int64)
    mat[:, 0] = pos
    mat[dsorted, 1 + rank] = pos[ssorted]

    idx1_maps, idx2_maps = [], []
    for c in range(NCORES):
        m = mat[c * NSH : (c + 1) * NSH].reshape(NT, P, W)
        m = np.ascontiguousarray(m.transpose(1, 0, 2).reshape(P, NT * W))
        idx1_maps.append((m * ROW1).astype(np.int32))
        idx2_maps.append((m * ROW2).astype(np.int32))

    bf = ml_dtypes.bfloat16
    W1aug = np.concatenate(
        [W1, (W1 @ a_src1)[:, None], (W1 @ a_dst1)[:, None]], axis=1
    ).astype(np.float32)
    w1s = (
        W1aug.reshape(KCH, P, AUG1).transpose(1, 0, 2).reshape(P, KCH * AUG1)
    ).astype(bf)
    v1 = np.concatenate([b1.astype(np.float32), np.zeros(2, np.float32)]).reshape(
        AUG1, 1
    )
    W2aug = np.concatenate(
        [W2, (W2 @ a_src2)[:, None], (W2 @ a_dst2)[:, None]], axis=1
    ).astype(np.float32)
    w2b = np.tile(W2aug.T.reshape(1, 4 * H), (P, 1)).astype(np.float32)
    b2a = np.tile(
        np.array([b2[0], b2[1], 0.0, 0.0], np.float32), (P, 1)
    ).astype(np.float32)

    xb = np.asarray(x, np.float32).astype(bf)
    in_maps = []
    for c in range(NCORES):
        in_maps.append(
            {
                "xs": np.ascontiguousarray(xb[NSH * c : NSH * (c + 1)].T),
                "w1s": w1s,
                "v1": v1,
                "idx1": idx1_maps[c],
                "idx2": idx2_maps[c],
                "w2b": w2b,
                "b2a": b2a,
            }
        )
    return W, in_maps


_NC_CACHE = {}


def _get_nc(W):
    if W not in _NC_CACHE:
        _NC_CACHE[W] = _build(W)
    return _NC_CACHE[W]


def kernel(**inputs):
    from concourse.bass_utils import run_bass_kernel_spmd

    W, in_maps = _prep(
        inputs["x"], inputs["edge_index"], inputs["W1"], inputs["a_src1"],
        inputs["a_dst1"], inputs["b1"], inputs["W2"], inputs["a_src2"],
        inputs["a_dst2"], inputs["b2"],
    )
    nc = _get_nc(W)
    res = run_bass_kernel_spmd(nc, in_maps, list(range(NCORES)))
    out = np.concatenate([res.results[c]["out"] for c in range(NCORES)], axis=0)
    out = out.astype(np.float64)
    mn, mx = out.min(), out.max()
    return (2.0 * (out - mn) / (mx - mn) - 1.0).astype(np.float32)
